# revision 30
# baseline (speedup 1.0000x reference)
"""Trainium2 Bass kernel for nn_Beta_cVAE (batch-parallel over 8 NeuronCores).

Layout: feature-major [feat, 128 batch] per core. The ADMM projection loop is
algebraically restructured so every trig/polar op collapses to rational form,
all linear terms fold into constant 11x11 matrices, and constraint terms are
computed in "h-form" (deviation from identity) so near-cancellations are exact.
Obstacle geometry is recentered on the iteration-0 trajectory so the position
subtractions are safe in bf16. Matmul-heavy parts run bf16 with fp32 PSUM; the
11-dim solve path stays fp32. The cVAE MLP runs batch-major with N=512 matmuls
and PE transposes between layers.
"""
import re
import numpy as np
import ml_dtypes

f32 = np.float32
bf16 = ml_dtypes.bfloat16

NC = 8
B = 1024
BL = B // NC                      # 128 batch rows per core
NUM, NVAR, NOBS = 100, 11, 10
T_FIN = 15.0
K_P = 20.0
K_D = 2.0 * np.sqrt(np.float32(20.0))
K_P_V = 20.0
A_OBS, B_OBS = 8.0, 4.2
RHO_V, RHO_PROJ, RHO_LANE, RHO_OBS, RHO_OFFSET, RHO_INEQ = 1.0, 1.0, 100.0, 100.0, 1.0, 100.0
V_MIN, V_MAX, A_MAX = 0.1, 30.0, 8.0
MAXITER = 20
HID = 1024
EPS_LN = 1e-12


def _kkt_inv(cost, A_eq):
    m = A_eq.shape[0]
    n = cost.shape[0]
    M = np.zeros((n + m, n + m), np.float64)
    M[:n, :n] = cost
    M[:n, n:] = A_eq.T
    M[n:, :n] = A_eq
    return np.linalg.inv(M).astype(f32)


def _install_drain_patch():
    """This walrus build allows only one sem-wait per CTRL instruction, but
    TileContext's exit drain attaches one wait per live sem. Split them."""
    import bass_rust
    from concourse.vector_clock import ScopedClock
    from concourse.tile import TileContext

    if getattr(TileContext, "_drain_patch_installed", False):
        return

    def _split_drain_and_barrier(self, tick_clock, wait_clock):
        nc = self.nc
        m = re.match(r"VectorClock\((\[.*\])\)", repr(tick_clock.global_clock))
        vals = eval(m.group(1))
        for i, v in enumerate(vals):
            if v > 0:
                single = [0] * len(vals)
                single[i] = v
                inst = nc.sync.nop()
                wait_clock.add_sem_waits(
                    inst.ins, ScopedClock({None: bass_rust.VectorClock(single)})
                )
        nc.sync.drain()
        nc.all_engine_barrier()
        assert self.sems is not None
        popped = nc._tile_sem_poison_stack.pop()
        assert popped is self._sem_poison
        nc.clear_and_free_semaphores(list(self.sems.allocated().values()))

    TileContext._drain_and_barrier = _split_drain_and_barrier
    TileContext._drain_patch_installed = True


def _split_multi_waits(nc):
    """Walrus on this image rejects instructions carrying more than one sem
    wait. Hoist extra waits onto single-wait NOPs just before the instruction
    on the same engine (same-engine program order makes this equivalent)."""
    import concourse.mybir as mybir
    import bass_rust

    n_nop = 0
    for fn in nc.m.functions:
        for blk in fn.blocks:
            insts = blk.instructions
            new_list = []
            changed = False
            for inst in insts:
                si = inst.sync_info
                if si is not None and len(si.on_wait) > 1:
                    waits = list(si.on_wait)
                    for w in waits[:-1]:
                        nop = mybir.InstNoOp(
                            name=f"wsplit_{n_nop}", ins=[], outs=[])
                        n_nop += 1
                        nop.engine = inst.engine
                        nop.sync_info = bass_rust.SyncInfo(
                            on_wait=[w], on_update=[])
                        new_list.append(nop)
                    inst.sync_info = bass_rust.SyncInfo(
                        on_wait=[waits[-1]], on_update=list(si.on_update))
                    changed = True
                new_list.append(inst)
            if changed:
                insts[:] = new_list
    return n_nop


# ---------------------------------------------------------------------------
# Host-side constant folding
# ---------------------------------------------------------------------------

def _host_prep(inputs):
    inp = np.asarray(inputs["inp"], f32)
    P = np.asarray(inputs["P"], f32)
    Pdot = np.asarray(inputs["Pdot"], f32)
    Pddot = np.asarray(inputs["Pddot"], f32)
    ise = np.asarray(inputs["initial_state_ego"], f32)
    t = np.linspace(0.0, T_FIN, NUM).astype(f32)

    A_eq_x = np.stack([P[0], Pdot[0], Pddot[0]]).astype(f32)
    A_eq_y = np.stack([P[0], Pdot[0], Pddot[0], Pdot[-1]]).astype(f32)
    A_pd = (Pddot - f32(K_P) * P - f32(K_D) * Pdot).astype(f32)
    A_vd = (Pddot - f32(K_P_V) * Pdot).astype(f32)
    cost_smooth = Pddot.T @ Pddot
    I = np.eye(NVAR, dtype=f32)

    inv1_x = _kkt_inv(cost_smooth + f32(RHO_V) * (A_vd.T @ A_vd), A_eq_x)
    inv1_y = _kkt_inv(cost_smooth + f32(RHO_OFFSET) * (A_pd.T @ A_pd), A_eq_y)
    A_obs_m = np.tile(P, (NOBS, 1))
    A_lane = np.vstack([P, -P]).astype(f32)
    cost2_x = (f32(RHO_PROJ) * I + f32(RHO_OBS) * (A_obs_m.T @ A_obs_m)
               + f32(RHO_INEQ) * (Pddot.T @ Pddot) + f32(RHO_INEQ) * (Pdot.T @ Pdot))
    cost2_y = cost2_x + f32(RHO_LANE) * (A_lane.T @ A_lane)
    inv2_x = _kkt_inv(cost2_x, A_eq_x)
    inv2_y = _kkt_inv(cost2_y, A_eq_y)
    Jx = inv2_x[:NVAR, :NVAR]
    Jy = inv2_y[:NVAR, :NVAR]
    J1x = inv1_x[:NVAR, :NVAR]
    J1y = inv1_y[:NVAR, :NVAR]

    G_pp = P.T @ P
    G_vv = Pdot.T @ Pdot
    G_aa = Pddot.T @ Pddot
    Mfull_x = f32(10 * RHO_OBS) * G_pp + f32(RHO_INEQ) * (G_aa + G_vv)
    Mfull_y = Mfull_x + f32(2 * RHO_LANE) * G_pp
    JGx = (Jx @ Mfull_x).astype(f32)
    JGy = (Jy @ Mfull_y).astype(f32)

    Rv = np.zeros((8, NUM), f32)
    Ry = np.zeros((8, NUM), f32)
    for k in range(4):
        Rv[k, 25 * k:25 * (k + 1)] = 1.0
        Ry[4 + k, 25 * k:25 * (k + 1)] = 1.0
    F_x = f32(-RHO_V * K_P_V) * (J1x @ (A_vd.T @ Rv.T))
    F_y = f32(-RHO_OFFSET * K_P) * (J1y @ (A_pd.T @ Ry.T))
    Gx = f32(RHO_PROJ) * (Jx @ F_x)
    Gy = f32(RHO_PROJ) * (Jy @ F_y)

    zb = np.zeros((B,), f32)
    beqx = np.stack([zb, ise[:, 2], zb], 1)
    beqy = np.stack([zb, ise[:, 3], zb, zb], 1)
    bx1 = inv1_x[:NVAR, NVAR:] @ beqx.T            # [11,B]
    by1 = inv1_y[:NVAR, NVAR:] @ beqy.T
    bx_c = inv2_x[:NVAR, NVAR:] @ beqx.T
    by_c = inv2_y[:NVAR, NVAR:] @ beqy.T
    DxC = f32(RHO_PROJ) * (Jx @ bx1) + bx_c
    DyC = f32(RHO_PROJ) * (Jy @ by1) + by_c

    # obstacle trajectories, feature-major, scaled: [100, 10, 2*B]
    x_obs, y_obs = inp[:, 5::5], inp[:, 6::5]
    vx_obs, vy_obs = inp[:, 7::5], inp[:, 8::5]
    xo = (x_obs[:, :, None] + vx_obs[:, :, None] * t).transpose(2, 1, 0)
    yo = (y_obs[:, :, None] + vy_obs[:, :, None] * t).transpose(2, 1, 0)
    xos = np.empty((NUM, NOBS, 2, B), f32)
    xos[:, :, 0, :] = f32(B_OBS) * xo
    xos[:, :, 1, :] = f32(A_OBS) * yo

    ubt = np.broadcast_to(f32(A_OBS) * np.asarray(inputs["y_ub"], f32), (NUM, B)).copy()
    lbt = np.broadcast_to(f32(A_OBS) * np.asarray(inputs["y_lb"], f32), (NUM, B)).copy()

    # MLP inputs feature-major
    inp_n = ((inp - np.asarray(inputs["inp_mean"], f32))
             / np.asarray(inputs["inp_std"], f32))
    enc_in0 = np.zeros((256, B), bf16)
    enc_in0[:55] = inp_n.T.astype(bf16)
    enc_in0[55:255] = np.asarray(inputs["traj_gt"], f32).T.astype(bf16)
    dec_base = np.zeros((57, B), bf16)
    dec_base[2:57] = inp_n.T.astype(bf16)
    epsT = np.asarray(inputs["eps"], f32).T.copy()   # [2,B]

    # MLP weights: [ko, no, 128, 512] chunks, bf16 (batch-major matmuls)
    w1 = np.asarray(inputs["enc_w1"], f32)
    w1p = np.zeros((256, HID), f32)
    w1p[:255] = w1
    w1n = np.ascontiguousarray(w1p.reshape(2, 128, HID)).astype(bf16)
    w2n = np.ascontiguousarray(
        np.asarray(inputs["enc_w2"], f32).reshape(8, 128, HID)).astype(bf16)
    d1n = np.ascontiguousarray(
        np.asarray(inputs["dec_w1"], f32).reshape(1, 57, HID)).astype(bf16)
    d2n = np.ascontiguousarray(
        np.asarray(inputs["dec_w2"], f32).reshape(8, 128, HID)).astype(bf16)
    wmu = np.ascontiguousarray(
        np.asarray(inputs["enc_wmu"], f32).reshape(8, 128, 2)).astype(bf16)
    wlv = np.ascontiguousarray(
        np.asarray(inputs["enc_wlv"], f32).reshape(8, 128, 2)).astype(bf16)
    w3c = np.ascontiguousarray(
        np.asarray(inputs["dec_w3"], f32).reshape(8, 128, 8)).astype(bf16)
    b1r = np.asarray(inputs["enc_b1"], f32).reshape(1, HID).astype(bf16)
    b2r = np.asarray(inputs["enc_b2"], f32).reshape(1, HID).astype(bf16)
    db1r = np.asarray(inputs["dec_b1"], f32).reshape(1, HID).astype(bf16)
    db2r = np.asarray(inputs["dec_b2"], f32).reshape(1, HID).astype(bf16)
    bmulv = np.stack([np.asarray(inputs["enc_bmu"], f32),
                      f32(0.5) * np.asarray(inputs["enc_blv"], f32)], 1)  # [2,2]
    db3 = np.asarray(inputs["dec_b3"], f32).reshape(8, 1).copy()
    id128 = np.eye(128, dtype=bf16)

    shared = {
        "PT": np.ascontiguousarray(P.T),
        "PdT": np.ascontiguousarray(Pdot.T),
        "PddT": np.ascontiguousarray(Pddot.T),
        "Lox": (f32(RHO_OBS / B_OBS) * P).astype(bf16),
        "Loy": (f32(RHO_OBS / A_OBS) * P).astype(bf16),
        "La": (f32(RHO_INEQ) * Pddot).astype(bf16),
        "Lv": (f32(RHO_INEQ) * Pdot).astype(bf16),
        "Ll": (f32(-RHO_LANE / A_OBS) * P).astype(bf16),
        "Jx": np.ascontiguousarray(Jx),      # symmetric -> its own lhsT
        "Jy": np.ascontiguousarray(Jy),
        "JGxT": np.ascontiguousarray(JGx.T),
        "JGyT": np.ascontiguousarray(JGy.T),
        "FxT": np.ascontiguousarray(F_x.T),  # [8,11]
        "FyT": np.ascontiguousarray(F_y.T),
        "GxT": np.ascontiguousarray(Gx.T),
        "GyT": np.ascontiguousarray(Gy.T),
        "w1n": w1n, "w2n": w2n, "wmu": wmu, "wlv": wlv,
        "d1n": d1n, "d2n": d2n, "w3c": w3c,
        "b1r": b1r, "b2r": b2r, "db1r": db1r, "db2r": db2r,
        "bmulv": bmulv, "db3": db3, "id128": id128,
    }
    per_core = []
    for c in range(NC):
        s = slice(c * BL, (c + 1) * BL)
        per_core.append({
            "enc_in": np.ascontiguousarray(
                np.concatenate([enc_in0[:128, s], enc_in0[128:256, s]], axis=1)),
            "dec_base": np.ascontiguousarray(dec_base[:, s]),
            "epsT": np.ascontiguousarray(epsT[:, s]),
            "xos": np.ascontiguousarray(xos[:, :, :, s].reshape(NUM, NOBS, 2 * BL)),
            "ubt": np.ascontiguousarray(ubt[:, s]),
            "lbt": np.ascontiguousarray(lbt[:, s]),
            "bx1": np.ascontiguousarray(bx1[:, s]),
            "by1": np.ascontiguousarray(by1[:, s]),
            "DxC": np.ascontiguousarray(DxC[:, s]),
            "DyC": np.ascontiguousarray(DyC[:, s]),
        })
    return shared, per_core


# ---------------------------------------------------------------------------
# Device program
# ---------------------------------------------------------------------------

_SHAPES = {
    "PT": ((NVAR, NUM), f32), "PdT": ((NVAR, NUM), f32), "PddT": ((NVAR, NUM), f32),
    "Lox": ((NUM, NVAR), bf16), "Loy": ((NUM, NVAR), bf16),
    "La": ((NUM, NVAR), bf16), "Lv": ((NUM, NVAR), bf16), "Ll": ((NUM, NVAR), bf16),
    "Jx": ((NVAR, NVAR), f32), "Jy": ((NVAR, NVAR), f32),
    "JGxT": ((NVAR, NVAR), f32), "JGyT": ((NVAR, NVAR), f32),
    "FxT": ((8, NVAR), f32), "FyT": ((8, NVAR), f32),
    "GxT": ((8, NVAR), f32), "GyT": ((8, NVAR), f32),
    "w1n": ((2, 128, HID), bf16), "w2n": ((8, 128, HID), bf16),
    "wmu": ((8, 128, 2), bf16), "wlv": ((8, 128, 2), bf16),
    "d1n": ((1, 57, HID), bf16), "d2n": ((8, 128, HID), bf16),
    "w3c": ((8, 128, 8), bf16),
    "b1r": ((1, HID), bf16), "b2r": ((1, HID), bf16),
    "db1r": ((1, HID), bf16), "db2r": ((1, HID), bf16),
    "bmulv": ((2, 2), f32), "db3": ((8, 1), f32), "id128": ((128, 128), bf16),
    "enc_in": ((128, 2 * BL), bf16), "dec_base": ((57, BL), bf16),
    "epsT": ((2, BL), f32), "xos": ((NUM, NOBS, 2 * BL), f32),
    "ubt": ((NUM, BL), f32), "lbt": ((NUM, BL), f32),
    "bx1": ((NVAR, BL), f32), "by1": ((NVAR, BL), f32),
    "DxC": ((NVAR, BL), f32), "DyC": ((NVAR, BL), f32),
}


def _build_program():
    import concourse.bass as bass
    import concourse.mybir as mybir
    from concourse.tile import TileContext

    _install_drain_patch()
    dt = mybir.dt
    DT = {np.dtype(f32): dt.float32, np.dtype(bf16): dt.bfloat16}
    Alu = mybir.AluOpType
    Act = mybir.ActivationFunctionType

    nc = bass.Bass("TRN2", target_bir_lowering=False, debug=False, num_devices=NC)
    d = {}
    for name, (shape, dtype) in _SHAPES.items():
        d[name] = nc.dram_tensor(name, list(shape), DT[np.dtype(dtype)],
                                 kind="ExternalInput").ap()
    out_d = nc.dram_tensor("out", [2, NVAR, BL], dt.float32,
                           kind="ExternalOutput").ap()

    lnAB = float(np.log(f32(A_OBS * B_OBS)))
    lnVMAX = float(np.log(f32(V_MAX)))
    lnVMIN = float(np.log(f32(V_MIN)))
    lnAMAX = float(np.log(f32(A_MAX)))

    with TileContext(nc) as tc:
        with tc.tile_pool(name="consts", bufs=1) as cpool, \
             tc.tile_pool(name="obs", bufs=2) as opool, \
             tc.tile_pool(name="state", bufs=2) as spool:

            # ================= MLP (batch-major, N=512) =================
            with tc.tile_pool(name="wstream", bufs=5) as wpool, \
                 tc.tile_pool(name="acts", bufs=2) as apool, \
                 tc.tile_pool(name="mlpps", bufs=2, space="PSUM") as mpp, \
                 tc.tile_pool(name="trps", bufs=2, space="PSUM") as tpp:

                enc_in = cpool.tile([128, 2 * BL], dt.bfloat16, tag="c_enc_in")
                nc.sync.dma_start(out=enc_in[:], in_=d["enc_in"][:])
                id128 = cpool.tile([128, 128], dt.bfloat16, tag="c_id")
                nc.sync.dma_start(out=id128[:], in_=d["id128"][:])
                ones1 = cpool.tile([1, 128], dt.bfloat16, tag="c_ones")
                nc.vector.memset(ones1[:], 1.0)
                brow = {}
                for nm in ("b1r", "b2r", "db1r", "db2r"):
                    brow[nm] = cpool.tile([1, HID], dt.bfloat16, tag=f"c_{nm}", name=f"brow_{nm}")
                    nc.sync.dma_start(out=brow[nm][:], in_=d[nm][:])
                bmulvt = cpool.tile([2, 2], dt.float32, tag="c_bmulv")
                nc.sync.dma_start(out=bmulvt[:], in_=d["bmulv"][:])
                db3t = cpool.tile([8, 1], dt.float32, tag="c_db3")
                nc.sync.dma_start(out=db3t[:], in_=d["db3"][:])

                def dense_bm(in_fm, w_dram, n_ko, brow_t, tagp):
                    """in_fm: list of [K,128] lhsT chunk APs. Returns 8 fm
                    chunks [128,128] bf16 of relu(in @ W + b): bias via K=1
                    ones outer product, two bm psums accumulated ko-major with
                    one [K,1024] weight DMA per ko (alternating DMA queues),
                    PE transpose back to fm."""
                    ps = []
                    for no in range(2):
                        p = mpp.tile([128, 512], dt.float32, tag=f"bm_ps{no}",
                                     name=f"bmps_{tagp}{no}", bufs=1)
                        nc.tensor.matmul(p[:], ones1[:],
                                         brow_t[:, no * 512:(no + 1) * 512],
                                         start=True, stop=False)
                        ps.append(p)
                    for ko in range(n_ko):
                        wt = wpool.tile([w_dram.shape[-2], HID],
                                        dt.bfloat16, tag=f"w_{tagp}")
                        eng = nc.sync if ko % 2 == 0 else nc.gpsimd
                        eng.dma_start(out=wt[:], in_=w_dram[ko])
                        for no in range(2):
                            nc.tensor.matmul(
                                ps[no][:], in_fm[ko],
                                wt[:, no * 512:(no + 1) * 512],
                                start=False, stop=(ko == n_ko - 1))
                    fm_out = []
                    for no in range(2):
                        hbm = apool.tile([128, 512], dt.bfloat16,
                                         tag=f"hbm_{tagp}{no}")
                        nc.scalar.activation(hbm[:], ps[no][:], Act.Relu)
                        for j in range(4):
                            pst = tpp.tile([128, 128], dt.bfloat16,
                                           tag="tr_ps", bufs=3,
                                           name=f"trps_{tagp}{no}{j}")
                            nc.tensor.transpose(
                                pst[:], hbm[:, j * 128:(j + 1) * 128], id128[:])
                            fm = apool.tile([128, 128], dt.bfloat16,
                                            tag=f"fm_{tagp}{no}{j}",
                                            name=f"fm_{tagp}{no}{j}")
                            nc.vector.tensor_copy(fm[:], pst[:])
                            fm_out.append(fm)
                    return fm_out

                h1 = dense_bm([enc_in[:, 0:BL], enc_in[:, BL:2 * BL]],
                              d["w1n"], 2, brow["b1r"], "e1")
                h2 = dense_bm([h[:] for h in h1], d["w2n"], 8, brow["b2r"], "e2")

                ps_mu = tpp.tile([2, BL], dt.float32, tag="mu", bufs=1)
                ps_lv = tpp.tile([2, BL], dt.float32, tag="lv", bufs=1)
                for ko in range(8):
                    wmut = wpool.tile([128, 2], dt.bfloat16, tag="wmu")
                    (nc.sync if ko % 2 == 0 else nc.gpsimd).dma_start(
                        out=wmut[:], in_=d["wmu"][ko])
                    nc.tensor.matmul(ps_mu[:], wmut[:], h2[ko][:],
                                     start=(ko == 0), stop=(ko == 7))
                    wlvt = wpool.tile([128, 2], dt.bfloat16, tag="wlv")
                    (nc.gpsimd if ko % 2 == 0 else nc.sync).dma_start(
                        out=wlvt[:], in_=d["wlv"][ko])
                    nc.tensor.matmul(ps_lv[:], wlvt[:], h2[ko][:],
                                     start=(ko == 0), stop=(ko == 7))
                elv = apool.tile([2, BL], dt.float32, tag="elv")
                nc.scalar.activation(elv[:], ps_lv[:], Act.Exp,
                                     bias=bmulvt[:, 1:2], scale=0.5)
                mu = apool.tile([2, BL], dt.float32, tag="mu_s")
                nc.scalar.activation(mu[:], ps_mu[:], Act.Identity,
                                     bias=bmulvt[:, 0:1], scale=1.0)
                epst = cpool.tile([2, BL], dt.float32, tag="c_eps")
                nc.sync.dma_start(out=epst[:], in_=d["epsT"][:])
                dec_in = cpool.tile([57, BL], dt.bfloat16, tag="c_dec_in")
                nc.sync.dma_start(out=dec_in[:], in_=d["dec_base"][:])
                ze = apool.tile([2, BL], dt.float32, tag="ze")
                nc.vector.tensor_tensor(ze[:], elv[:], epst[:], Alu.mult)
                nc.vector.tensor_tensor(dec_in[0:2, :], ze[:], mu[:], Alu.add)

                g1 = dense_bm([dec_in[:]], d["d1n"], 1, brow["db1r"], "d1")
                g2 = dense_bm([g[:] for g in g1], d["d2n"], 8, brow["db2r"], "d2")

                ps_nn = tpp.tile([8, BL], dt.float32, tag="nn", bufs=1)
                for ko in range(8):
                    w3t = wpool.tile([128, 8], dt.bfloat16, tag="w3")
                    (nc.sync if ko % 2 == 0 else nc.gpsimd).dma_start(
                        out=w3t[:], in_=d["w3c"][ko])
                    nc.tensor.matmul(ps_nn[:], w3t[:], g2[ko][:],
                                     start=(ko == 0), stop=(ko == 7))
                nn = cpool.tile([8, BL], dt.float32, tag="c_nn")
                nc.scalar.activation(nn[:], ps_nn[:], Act.Identity,
                                     bias=db3t[:], scale=1.0)

            # ================= setup + ADMM loop =================
            with tc.tile_pool(name="fwdps", bufs=4, space="PSUM") as fps, \
                 tc.tile_pool(name="chainps", bufs=4, space="PSUM") as cps:

                def ctile(name, eng=None):
                    shape = list(_SHAPES[name][0])
                    dtype = DT[np.dtype(_SHAPES[name][1])]
                    t = cpool.tile(shape, dtype, tag=f"c_{name}", name=f"ct_{name}")
                    (eng or nc.sync).dma_start(out=t[:], in_=d[name][:])
                    return t

                PTt = ctile("PT"); PdTt = ctile("PdT"); PddTt = ctile("PddT")
                Loxt = ctile("Lox"); Loyt = ctile("Loy")
                Lat = ctile("La"); Lvt = ctile("Lv"); Llt = ctile("Ll")
                Jxt = ctile("Jx"); Jyt = ctile("Jy")
                JGxTt = ctile("JGxT"); JGyTt = ctile("JGyT")
                FxTt = ctile("FxT"); FyTt = ctile("FyT")
                GxTt = ctile("GxT"); GyTt = ctile("GyT")
                xost = ctile("xos", eng=nc.gpsimd)
                ubtt = ctile("ubt"); lbtt = ctile("lbt")
                bx1t = ctile("bx1"); by1t = ctile("by1")
                DxCt = ctile("DxC"); DyCt = ctile("DyC")

                cbias = cpool.tile([128, 6], dt.float32, tag="c_cbias")
                nc.vector.memset(cbias[:, 0:1], EPS_LN)
                nc.vector.memset(cbias[:, 1:2], lnAB)
                nc.vector.memset(cbias[:, 2:3], -lnVMAX)
                nc.vector.memset(cbias[:, 3:4], lnVMIN)
                nc.vector.memset(cbias[:, 4:5], -lnAMAX)

                # prim0 / Dx2 / Dy2
                ps0 = cps.tile([NVAR, BL], dt.float32, tag="chain")
                prim0 = cpool.tile([NVAR, 2 * BL], dt.float32, tag="c_prim0")
                nc.tensor.matmul(ps0[:], FxTt[:], nn[:], start=True, stop=True)
                nc.vector.tensor_tensor(prim0[:, 0:BL], ps0[:], bx1t[:], Alu.add)
                ps1 = cps.tile([NVAR, BL], dt.float32, tag="chain")
                nc.tensor.matmul(ps1[:], FyTt[:], nn[:], start=True, stop=True)
                nc.vector.tensor_tensor(prim0[:, BL:2 * BL], ps1[:], by1t[:],
                                        Alu.add)
                Dx2 = cpool.tile([NVAR, BL], dt.float32, tag="c_Dx2")
                ps2 = cps.tile([NVAR, BL], dt.float32, tag="chain")
                nc.tensor.matmul(ps2[:], GxTt[:], nn[:], start=True, stop=True)
                nc.vector.tensor_tensor(Dx2[:], ps2[:], DxCt[:], Alu.add)
                Dy2 = cpool.tile([NVAR, BL], dt.float32, tag="c_Dy2")
                ps3 = cps.tile([NVAR, BL], dt.float32, tag="chain")
                nc.tensor.matmul(ps3[:], GyTt[:], nn[:], start=True, stop=True)
                nc.vector.tensor_tensor(Dy2[:], ps3[:], DyCt[:], Alu.add)

                # recentering: xy0 (scaled), shifted obstacle / lane tiles
                ps_xy0 = fps.tile([NUM, 2 * BL], dt.float32, tag="fwd")
                nc.tensor.matmul(ps_xy0[:], PTt[:], prim0[:],
                                 start=True, stop=True)
                xy0s = cpool.tile([NUM, 2 * BL], dt.float32, tag="c_xy0s")
                nc.scalar.activation(xy0s[:, 0:BL], ps_xy0[:, 0:BL], Act.Copy,
                                     scale=float(B_OBS))
                nc.scalar.activation(xy0s[:, BL:2 * BL], ps_xy0[:, BL:2 * BL],
                                     Act.Copy, scale=float(A_OBS))
                xo_r = cpool.tile([NUM, NOBS, 2 * BL], dt.bfloat16, tag="c_xo_r")
                nc.vector.tensor_tensor(
                    xo_r[:], xost[:],
                    xy0s[:, None, :].to_broadcast((NUM, NOBS, 2 * BL)),
                    Alu.subtract)
                ub_r = cpool.tile([NUM, BL], dt.float32, tag="c_ub_r")
                nc.vector.tensor_tensor(ub_r[:], ubtt[:], xy0s[:, BL:2 * BL],
                                        Alu.subtract)
                lb_r = cpool.tile([NUM, BL], dt.float32, tag="c_lb_r")
                nc.vector.tensor_tensor(lb_r[:], lbtt[:], xy0s[:, BL:2 * BL],
                                        Alu.subtract)

                DDx = cpool.tile([NVAR, BL], dt.float32, tag="c_DDx")
                nc.vector.tensor_tensor(DDx[:], Dx2[:], prim0[:, 0:BL],
                                        Alu.subtract)
                DDy = cpool.tile([NVAR, BL], dt.float32, tag="c_DDy")
                nc.vector.tensor_tensor(DDy[:], Dy2[:], prim0[:, BL:2 * BL],
                                        Alu.subtract)

                lx = spool.tile([NVAR, BL], dt.float32, tag="lx")
                ly = spool.tile([NVAR, BL], dt.float32, tag="ly")
                nc.vector.memset(lx[:], 0.0)
                nc.vector.memset(ly[:], 0.0)
                prim = prim0
                dprim = None

                # ---------- ADMM loop ----------
                for it in range(MAXITER):
                    ps_xyd = fps.tile([NUM, 2 * BL], dt.float32, tag="fwd")
                    nc.tensor.matmul(ps_xyd[:], PdTt[:], prim[:],
                                     start=True, stop=True)
                    ps_xydd = fps.tile([NUM, 2 * BL], dt.float32, tag="fwd")
                    nc.tensor.matmul(ps_xydd[:], PddTt[:], prim[:],
                                     start=True, stop=True)

                    # scaled recentered positions, bf16 [100, 256]
                    xys = opool.tile([NUM, 2 * BL], dt.bfloat16, tag="xys")
                    if it == 0:
                        nc.vector.memset(xys[:], 0.0)
                    else:
                        ps_xy = fps.tile([NUM, 2 * BL], dt.float32, tag="fwd")
                        nc.tensor.matmul(ps_xy[:], PTt[:], dprim[:],
                                         start=True, stop=True)
                        nc.scalar.activation(xys[:, 0:BL], ps_xy[:, 0:BL],
                                             Act.Copy, scale=float(B_OBS))
                        nc.scalar.activation(xys[:, BL:2 * BL],
                                             ps_xy[:, BL:2 * BL],
                                             Act.Copy, scale=float(A_OBS))

                    # obstacles: wc' = xys - xo_r (bf16, recentered),
                    # processed in two halves so stages pipeline
                    HOB = NOBS // 2
                    wcws = opool.tile([NUM, NOBS, 2 * BL], dt.bfloat16,
                                      tag="wcws")
                    sq = opool.tile([NUM, NOBS, 2 * BL], dt.bfloat16, tag="sq")
                    r2 = opool.tile([NUM, NOBS + 2, BL], dt.bfloat16, tag="r2")
                    for hh in range(2):
                        oh = slice(hh * HOB, (hh + 1) * HOB)
                        nc.vector.tensor_tensor(
                            wcws[:, oh, :],
                            xys[:, None, :].to_broadcast((NUM, HOB, 2 * BL)),
                            xo_r[:, oh, :], Alu.subtract)
                        nc.vector.tensor_tensor(sq[:, oh, 0:BL],
                                                wcws[:, oh, 0:BL],
                                                wcws[:, oh, 0:BL], Alu.mult)
                        nc.scalar.activation(sq[:, oh, BL:2 * BL],
                                             wcws[:, oh, BL:2 * BL], Act.Square)
                        nc.vector.tensor_tensor(r2[:, oh, :], sq[:, oh, 0:BL],
                                                sq[:, oh, BL:2 * BL], Alu.add)
                    # velocity / accel squares straight from PSUM
                    qd = opool.tile([NUM, 4 * BL], dt.bfloat16, tag="qd")
                    nc.scalar.activation(qd[:, 0:2 * BL], ps_xyd[:], Act.Square)
                    nc.scalar.activation(qd[:, 2 * BL:4 * BL], ps_xydd[:],
                                         Act.Square)
                    nc.vector.tensor_tensor(r2[:, NOBS, :], qd[:, 0:BL],
                                            qd[:, BL:2 * BL], Alu.add)
                    nc.vector.tensor_tensor(r2[:, NOBS + 1, :],
                                            qd[:, 2 * BL:3 * BL],
                                            qd[:, 3 * BL:4 * BL], Alu.add)

                    lnt = opool.tile([NUM, NOBS + 2, BL], dt.bfloat16, tag="lnt")
                    qob = opool.tile([NUM, NOBS, BL], dt.bfloat16, tag="qob")
                    em = opool.tile([NUM, NOBS, BL], dt.bfloat16, tag="em")
                    m1 = opool.tile([NUM, NOBS, BL], dt.bfloat16, tag="m1")
                    uv = opool.tile([NUM, NOBS, 2 * BL], dt.bfloat16, tag="uv")
                    for hh in range(2):
                        oh = slice(hh * HOB, (hh + 1) * HOB)
                        nc.scalar.activation(lnt[:, oh, :], r2[:, oh, :],
                                             Act.Ln, bias=cbias[0:NUM, 0:1])
                        # m1-1 = max(exp(-ln/2 + lnAB), 1) - 1 (clamps on DVE)
                        nc.vector.tensor_scalar(qob[:, oh, :], lnt[:, oh, :],
                                                -0.5, lnAB, Alu.mult, Alu.add)
                        nc.scalar.activation(em[:, oh, :], qob[:, oh, :],
                                             Act.Exp)
                        nc.vector.tensor_scalar(m1[:, oh, :], em[:, oh, :],
                                                1.0, 1.0, Alu.max, Alu.subtract)
                        nc.vector.tensor_tensor(uv[:, oh, 0:BL], m1[:, oh, :],
                                                wcws[:, oh, 0:BL], Alu.mult)
                        nc.vector.tensor_tensor(uv[:, oh, BL:2 * BL],
                                                m1[:, oh, :],
                                                wcws[:, oh, BL:2 * BL], Alu.mult)
                    nc.scalar.activation(lnt[:, NOBS:NOBS + 2, :],
                                         r2[:, NOBS:NOBS + 2, :],
                                         Act.Ln, bias=cbias[0:NUM, 0:1])

                    # velocity: gv = max(min(Vmax/rv,1), Vmin/rv)
                    qv = opool.tile([NUM, BL], dt.bfloat16, tag="qv")
                    nc.scalar.activation(qv[:], lnt[:, NOBS, :], Act.Relu,
                                         bias=cbias[0:NUM, 2:3], scale=0.5)
                    gva = opool.tile([NUM, BL], dt.bfloat16, tag="gva")
                    nc.scalar.activation(gva[:], qv[:], Act.Exp, scale=-1.0)
                    gvb = opool.tile([NUM, BL], dt.bfloat16, tag="gvb")
                    nc.scalar.activation(gvb[:], lnt[:, NOBS, :], Act.Exp,
                                         bias=cbias[0:NUM, 3:4], scale=-0.5)
                    gv = opool.tile([NUM, BL], dt.bfloat16, tag="gv")
                    nc.vector.tensor_tensor(gv[:], gva[:], gvb[:], Alu.max)
                    uvd = opool.tile([NUM, 2, BL], dt.bfloat16, tag="uvd")
                    nc.vector.scalar_tensor_tensor(
                        uvd[:], gv[:, None, :].to_broadcast((NUM, 2, BL)),
                        1.0, ps_xyd[:].rearrange("p (c b) -> p c b", c=2),
                        Alu.subtract, Alu.mult)

                    # accel: ga = min(Amax/ra, 1)
                    qa = opool.tile([NUM, BL], dt.bfloat16, tag="qa")
                    nc.scalar.activation(qa[:], lnt[:, NOBS + 1, :], Act.Relu,
                                         bias=cbias[0:NUM, 4:5], scale=0.5)
                    ga = opool.tile([NUM, BL], dt.bfloat16, tag="ga")
                    nc.scalar.activation(ga[:], qa[:], Act.Exp, scale=-1.0)
                    uad = opool.tile([NUM, 2, BL], dt.bfloat16, tag="uad")
                    nc.vector.scalar_tensor_tensor(
                        uad[:], ga[:, None, :].to_broadcast((NUM, 2, BL)),
                        1.0, ps_xydd[:].rearrange("p (c b) -> p c b", c=2),
                        Alu.subtract, Alu.mult)

                    # lane (recentered, A-scaled)
                    c1 = opool.tile([NUM, BL], dt.float32, tag="c1")
                    nc.vector.tensor_tensor(c1[:], xys[:, BL:2 * BL], lb_r[:],
                                            Alu.max)
                    c2 = opool.tile([NUM, BL], dt.float32, tag="c2")
                    nc.vector.tensor_tensor(c2[:], c1[:], ub_r[:], Alu.min)
                    elane = opool.tile([NUM, BL], dt.bfloat16, tag="elane")
                    nc.vector.tensor_tensor(elane[:], xys[:, BL:2 * BL], c2[:],
                                            Alu.subtract)

                    # T chains
                    ps_tx = cps.tile([NVAR, BL], dt.float32, tag="chain")
                    for o in range(NOBS):
                        nc.tensor.matmul(ps_tx[:], Loxt[:], uv[:, o, 0:BL],
                                         start=(o == 0), stop=False)
                    nc.tensor.matmul(ps_tx[:], Lvt[:], uvd[:, 0, :],
                                     start=False, stop=False)
                    nc.tensor.matmul(ps_tx[:], Lat[:], uad[:, 0, :],
                                     start=False, stop=True)
                    ps_ty = cps.tile([NVAR, BL], dt.float32, tag="chain")
                    for o in range(NOBS):
                        nc.tensor.matmul(ps_ty[:], Loyt[:], uv[:, o, BL:2 * BL],
                                         start=(o == 0), stop=False)
                    nc.tensor.matmul(ps_ty[:], Lvt[:], uvd[:, 1, :],
                                     start=False, stop=False)
                    nc.tensor.matmul(ps_ty[:], Lat[:], uad[:, 1, :],
                                     start=False, stop=False)
                    nc.tensor.matmul(ps_ty[:], Llt[:], elane[:],
                                     start=False, stop=True)

                    # z = lx + 2*T  (J z = J lx_new + J T), straight from PSUM
                    z_x = opool.tile([NVAR, BL], dt.float32, tag="z_x")
                    nc.vector.scalar_tensor_tensor(
                        z_x[:], ps_tx[:], 2.0, lx[:], Alu.mult, Alu.add)
                    z_y = opool.tile([NVAR, BL], dt.float32, tag="z_y")
                    nc.vector.scalar_tensor_tensor(
                        z_y[:], ps_ty[:], 2.0, ly[:], Alu.mult, Alu.add)
                    lx_n = spool.tile([NVAR, BL], dt.float32, tag="lx")
                    nc.vector.tensor_tensor(lx_n[:], lx[:], ps_tx[:], Alu.add)
                    ly_n = spool.tile([NVAR, BL], dt.float32, tag="ly")
                    nc.vector.tensor_tensor(ly_n[:], ly[:], ps_ty[:], Alu.add)

                    ps_sx = cps.tile([NVAR, BL], dt.float32, tag="chain")
                    nc.tensor.matmul(ps_sx[:], Jxt[:], z_x[:],
                                     start=True, stop=False)
                    nc.tensor.matmul(ps_sx[:], JGxTt[:], prim[:, 0:BL],
                                     start=False, stop=True)
                    ps_sy = cps.tile([NVAR, BL], dt.float32, tag="chain")
                    nc.tensor.matmul(ps_sy[:], Jyt[:], z_y[:],
                                     start=True, stop=False)
                    nc.tensor.matmul(ps_sy[:], JGyTt[:], prim[:, BL:2 * BL],
                                     start=False, stop=True)

                    prim_n = spool.tile([NVAR, 2 * BL], dt.float32, tag="prim")
                    nc.vector.tensor_tensor(prim_n[:, 0:BL], ps_sx[:], Dx2[:],
                                            Alu.add)
                    nc.vector.tensor_tensor(prim_n[:, BL:2 * BL], ps_sy[:],
                                            Dy2[:], Alu.add)

                    if it + 1 < MAXITER:
                        dprim_n = spool.tile([NVAR, 2 * BL], dt.float32,
                                             tag="dprim")
                        nc.vector.tensor_tensor(dprim_n[:, 0:BL], ps_sx[:],
                                                DDx[:], Alu.add)
                        nc.vector.tensor_tensor(dprim_n[:, BL:2 * BL], ps_sy[:],
                                                DDy[:], Alu.add)
                        dprim = dprim_n
                    prim, lx, ly = prim_n, lx_n, ly_n

                # ---------- output ----------
                nc.sync.dma_start(out=out_d[0], in_=prim[:, 0:BL])
                nc.sync.dma_start(out=out_d[1], in_=prim[:, BL:2 * BL])

    _split_multi_waits(nc)
    return nc


# ---------------------------------------------------------------------------
# Entry point
# ---------------------------------------------------------------------------

def kernel(**inputs):
    from concourse.bass_utils import run_bass_kernel_spmd

    shared, per_core = _host_prep(inputs)
    nc = _build_program()
    in_maps = [{**shared, **pc} for pc in per_core]
    res = run_bass_kernel_spmd(nc, in_maps, list(range(NC)))
    out = np.empty((B, 2 * NVAR), f32)
    for c in range(NC):
        r = res.results[c]["out"]
        out[c * BL:(c + 1) * BL, 0:NVAR] = r[0].T
        out[c * BL:(c + 1) * BL, NVAR:2 * NVAR] = r[1].T
    return out


# revision 32
# speedup vs baseline: 11.5639x; 11.5639x over previous
"""Trainium2 Bass kernel for nn_Beta_cVAE (batch-parallel over 8 NeuronCores).

Layout: feature-major [feat, 128 batch] per core. The ADMM projection loop is
algebraically restructured so every trig/polar op collapses to rational form,
all linear terms fold into constant 11x11 matrices, and constraint terms are
computed in "h-form" (deviation from identity) so near-cancellations are exact.
Obstacle geometry is recentered on the iteration-0 trajectory so the position
subtractions are safe in bf16. Matmul-heavy parts run bf16 with fp32 PSUM; the
11-dim solve path stays fp32. The cVAE MLP runs batch-major with N=512 matmuls
and PE transposes between layers.
"""
import re
import numpy as np
import ml_dtypes

f32 = np.float32
bf16 = ml_dtypes.bfloat16

NC = 8
B = 1024
BL = B // NC                      # 128 batch rows per core
NUM, NVAR, NOBS = 100, 11, 10
T_FIN = 15.0
K_P = 20.0
K_D = 2.0 * np.sqrt(np.float32(20.0))
K_P_V = 20.0
A_OBS, B_OBS = 8.0, 4.2
RHO_V, RHO_PROJ, RHO_LANE, RHO_OBS, RHO_OFFSET, RHO_INEQ = 1.0, 1.0, 100.0, 100.0, 1.0, 100.0
V_MIN, V_MAX, A_MAX = 0.1, 30.0, 8.0
MAXITER = 20
HID = 1024
EPS_LN = 1e-12


def _kkt_inv(cost, A_eq):
    m = A_eq.shape[0]
    n = cost.shape[0]
    M = np.zeros((n + m, n + m), np.float64)
    M[:n, :n] = cost
    M[:n, n:] = A_eq.T
    M[n:, :n] = A_eq
    return np.linalg.inv(M).astype(f32)


def _install_drain_patch():
    """This walrus build allows only one sem-wait per CTRL instruction, but
    TileContext's exit drain attaches one wait per live sem. Split them."""
    import bass_rust
    from concourse.vector_clock import ScopedClock
    from concourse.tile import TileContext

    if getattr(TileContext, "_drain_patch_installed", False):
        return

    def _split_drain_and_barrier(self, tick_clock, wait_clock):
        nc = self.nc
        m = re.match(r"VectorClock\((\[.*\])\)", repr(tick_clock.global_clock))
        vals = eval(m.group(1))
        for i, v in enumerate(vals):
            if v > 0:
                single = [0] * len(vals)
                single[i] = v
                inst = nc.sync.nop()
                wait_clock.add_sem_waits(
                    inst.ins, ScopedClock({None: bass_rust.VectorClock(single)})
                )
        nc.sync.drain()
        nc.all_engine_barrier()
        assert self.sems is not None
        popped = nc._tile_sem_poison_stack.pop()
        assert popped is self._sem_poison
        nc.clear_and_free_semaphores(list(self.sems.allocated().values()))

    TileContext._drain_and_barrier = _split_drain_and_barrier
    TileContext._drain_patch_installed = True


def _split_multi_waits(nc):
    """Walrus on this image rejects instructions carrying more than one sem
    wait. Hoist extra waits onto single-wait NOPs just before the instruction
    on the same engine (same-engine program order makes this equivalent)."""
    import concourse.mybir as mybir
    import bass_rust

    n_nop = 0
    for fn in nc.m.functions:
        for blk in fn.blocks:
            insts = blk.instructions
            new_list = []
            changed = False
            for inst in insts:
                si = inst.sync_info
                if si is not None and len(si.on_wait) > 1:
                    waits = list(si.on_wait)
                    for w in waits[:-1]:
                        nop = mybir.InstNoOp(
                            name=f"wsplit_{n_nop}", ins=[], outs=[])
                        n_nop += 1
                        nop.engine = inst.engine
                        nop.sync_info = bass_rust.SyncInfo(
                            on_wait=[w], on_update=[])
                        new_list.append(nop)
                    inst.sync_info = bass_rust.SyncInfo(
                        on_wait=[waits[-1]], on_update=list(si.on_update))
                    changed = True
                new_list.append(inst)
            if changed:
                insts[:] = new_list
    return n_nop


# ---------------------------------------------------------------------------
# Host-side constant folding
# ---------------------------------------------------------------------------

def _host_prep(inputs):
    inp = np.asarray(inputs["inp"], f32)
    P = np.asarray(inputs["P"], f32)
    Pdot = np.asarray(inputs["Pdot"], f32)
    Pddot = np.asarray(inputs["Pddot"], f32)
    ise = np.asarray(inputs["initial_state_ego"], f32)
    t = np.linspace(0.0, T_FIN, NUM).astype(f32)

    A_eq_x = np.stack([P[0], Pdot[0], Pddot[0]]).astype(f32)
    A_eq_y = np.stack([P[0], Pdot[0], Pddot[0], Pdot[-1]]).astype(f32)
    A_pd = (Pddot - f32(K_P) * P - f32(K_D) * Pdot).astype(f32)
    A_vd = (Pddot - f32(K_P_V) * Pdot).astype(f32)
    cost_smooth = Pddot.T @ Pddot
    I = np.eye(NVAR, dtype=f32)

    inv1_x = _kkt_inv(cost_smooth + f32(RHO_V) * (A_vd.T @ A_vd), A_eq_x)
    inv1_y = _kkt_inv(cost_smooth + f32(RHO_OFFSET) * (A_pd.T @ A_pd), A_eq_y)
    A_obs_m = np.tile(P, (NOBS, 1))
    A_lane = np.vstack([P, -P]).astype(f32)
    cost2_x = (f32(RHO_PROJ) * I + f32(RHO_OBS) * (A_obs_m.T @ A_obs_m)
               + f32(RHO_INEQ) * (Pddot.T @ Pddot) + f32(RHO_INEQ) * (Pdot.T @ Pdot))
    cost2_y = cost2_x + f32(RHO_LANE) * (A_lane.T @ A_lane)
    inv2_x = _kkt_inv(cost2_x, A_eq_x)
    inv2_y = _kkt_inv(cost2_y, A_eq_y)
    Jx = inv2_x[:NVAR, :NVAR]
    Jy = inv2_y[:NVAR, :NVAR]
    J1x = inv1_x[:NVAR, :NVAR]
    J1y = inv1_y[:NVAR, :NVAR]

    G_pp = P.T @ P
    G_vv = Pdot.T @ Pdot
    G_aa = Pddot.T @ Pddot
    Mfull_x = f32(10 * RHO_OBS) * G_pp + f32(RHO_INEQ) * (G_aa + G_vv)
    Mfull_y = Mfull_x + f32(2 * RHO_LANE) * G_pp
    JGx = (Jx @ Mfull_x).astype(f32)
    JGy = (Jy @ Mfull_y).astype(f32)

    Rv = np.zeros((8, NUM), f32)
    Ry = np.zeros((8, NUM), f32)
    for k in range(4):
        Rv[k, 25 * k:25 * (k + 1)] = 1.0
        Ry[4 + k, 25 * k:25 * (k + 1)] = 1.0
    F_x = f32(-RHO_V * K_P_V) * (J1x @ (A_vd.T @ Rv.T))
    F_y = f32(-RHO_OFFSET * K_P) * (J1y @ (A_pd.T @ Ry.T))
    Gx = f32(RHO_PROJ) * (Jx @ F_x)
    Gy = f32(RHO_PROJ) * (Jy @ F_y)

    zb = np.zeros((B,), f32)
    beqx = np.stack([zb, ise[:, 2], zb], 1)
    beqy = np.stack([zb, ise[:, 3], zb, zb], 1)
    bx1 = inv1_x[:NVAR, NVAR:] @ beqx.T            # [11,B]
    by1 = inv1_y[:NVAR, NVAR:] @ beqy.T
    bx_c = inv2_x[:NVAR, NVAR:] @ beqx.T
    by_c = inv2_y[:NVAR, NVAR:] @ beqy.T
    DxC = f32(RHO_PROJ) * (Jx @ bx1) + bx_c
    DyC = f32(RHO_PROJ) * (Jy @ by1) + by_c

    # obstacle trajectories, feature-major, scaled: [100, 10, 2*B]
    x_obs, y_obs = inp[:, 5::5], inp[:, 6::5]
    vx_obs, vy_obs = inp[:, 7::5], inp[:, 8::5]
    xo = (x_obs[:, :, None] + vx_obs[:, :, None] * t).transpose(2, 1, 0)
    yo = (y_obs[:, :, None] + vy_obs[:, :, None] * t).transpose(2, 1, 0)
    xos = np.empty((NUM, NOBS, 2, B), f32)
    xos[:, :, 0, :] = f32(B_OBS) * xo
    xos[:, :, 1, :] = f32(A_OBS) * yo

    ubt = np.broadcast_to(f32(A_OBS) * np.asarray(inputs["y_ub"], f32), (NUM, B)).copy()
    lbt = np.broadcast_to(f32(A_OBS) * np.asarray(inputs["y_lb"], f32), (NUM, B)).copy()

    # MLP inputs feature-major
    inp_n = ((inp - np.asarray(inputs["inp_mean"], f32))
             / np.asarray(inputs["inp_std"], f32))
    enc_in0 = np.zeros((256, B), bf16)
    enc_in0[:55] = inp_n.T.astype(bf16)
    enc_in0[55:255] = np.asarray(inputs["traj_gt"], f32).T.astype(bf16)
    dec_base = np.zeros((57, B), bf16)
    dec_base[2:57] = inp_n.T.astype(bf16)
    epsT = np.asarray(inputs["eps"], f32).T.copy()   # [2,B]

    # MLP weights: [ko, no, 128, 512] chunks, bf16 (batch-major matmuls)
    w1 = np.asarray(inputs["enc_w1"], f32)
    w1p = np.zeros((256, HID), f32)
    w1p[:255] = w1
    w1n = np.ascontiguousarray(w1p.reshape(2, 128, HID)).astype(bf16)
    w2n = np.ascontiguousarray(
        np.asarray(inputs["enc_w2"], f32).reshape(8, 128, HID)).astype(bf16)
    d1n = np.ascontiguousarray(
        np.asarray(inputs["dec_w1"], f32).reshape(1, 57, HID)).astype(bf16)
    d2n = np.ascontiguousarray(
        np.asarray(inputs["dec_w2"], f32).reshape(8, 128, HID)).astype(bf16)
    wmu = np.ascontiguousarray(
        np.asarray(inputs["enc_wmu"], f32).reshape(8, 128, 2)).astype(bf16)
    wlv = np.ascontiguousarray(
        np.asarray(inputs["enc_wlv"], f32).reshape(8, 128, 2)).astype(bf16)
    w3c = np.ascontiguousarray(
        np.asarray(inputs["dec_w3"], f32).reshape(8, 128, 8)).astype(bf16)
    b1r = np.asarray(inputs["enc_b1"], f32).reshape(1, HID).astype(bf16)
    b2r = np.asarray(inputs["enc_b2"], f32).reshape(1, HID).astype(bf16)
    db1r = np.asarray(inputs["dec_b1"], f32).reshape(1, HID).astype(bf16)
    db2r = np.asarray(inputs["dec_b2"], f32).reshape(1, HID).astype(bf16)
    bmulv = np.stack([np.asarray(inputs["enc_bmu"], f32),
                      f32(0.5) * np.asarray(inputs["enc_blv"], f32)], 1)  # [2,2]
    db3 = np.asarray(inputs["dec_b3"], f32).reshape(8, 1).copy()
    id128 = np.eye(128, dtype=bf16)

    shared = {
        "PT": np.ascontiguousarray(P.T),
        "PdT": np.ascontiguousarray(Pdot.T),
        "PddT": np.ascontiguousarray(Pddot.T),
        "Lox": (f32(RHO_OBS / B_OBS) * P).astype(bf16),
        "Loy": (f32(RHO_OBS / A_OBS) * P).astype(bf16),
        "La": (f32(RHO_INEQ) * Pddot).astype(bf16),
        "Lv": (f32(RHO_INEQ) * Pdot).astype(bf16),
        "Ll": (f32(-RHO_LANE / A_OBS) * P).astype(bf16),
        "Jx": np.ascontiguousarray(Jx),      # symmetric -> its own lhsT
        "Jy": np.ascontiguousarray(Jy),
        "JGxT": np.ascontiguousarray(JGx.T),
        "JGyT": np.ascontiguousarray(JGy.T),
        "FxT": np.ascontiguousarray(F_x.T),  # [8,11]
        "FyT": np.ascontiguousarray(F_y.T),
        "GxT": np.ascontiguousarray(Gx.T),
        "GyT": np.ascontiguousarray(Gy.T),
        "w1n": w1n, "w2n": w2n, "wmu": wmu, "wlv": wlv,
        "d1n": d1n, "d2n": d2n, "w3c": w3c,
        "b1r": b1r, "b2r": b2r, "db1r": db1r, "db2r": db2r,
        "bmulv": bmulv, "db3": db3, "id128": id128,
    }
    per_core = []
    for c in range(NC):
        s = slice(c * BL, (c + 1) * BL)
        per_core.append({
            "enc_in": np.ascontiguousarray(
                np.concatenate([enc_in0[:128, s], enc_in0[128:256, s]], axis=1)),
            "dec_base": np.ascontiguousarray(dec_base[:, s]),
            "epsT": np.ascontiguousarray(epsT[:, s]),
            "xos": np.ascontiguousarray(xos[:, :, :, s].reshape(NUM, NOBS, 2 * BL)),
            "ubt": np.ascontiguousarray(ubt[:, s]),
            "lbt": np.ascontiguousarray(lbt[:, s]),
            "bx1": np.ascontiguousarray(bx1[:, s]),
            "by1": np.ascontiguousarray(by1[:, s]),
            "DxC": np.ascontiguousarray(DxC[:, s]),
            "DyC": np.ascontiguousarray(DyC[:, s]),
        })
    return shared, per_core


# ---------------------------------------------------------------------------
# Device program
# ---------------------------------------------------------------------------

_SHAPES = {
    "PT": ((NVAR, NUM), f32), "PdT": ((NVAR, NUM), f32), "PddT": ((NVAR, NUM), f32),
    "Lox": ((NUM, NVAR), bf16), "Loy": ((NUM, NVAR), bf16),
    "La": ((NUM, NVAR), bf16), "Lv": ((NUM, NVAR), bf16), "Ll": ((NUM, NVAR), bf16),
    "Jx": ((NVAR, NVAR), f32), "Jy": ((NVAR, NVAR), f32),
    "JGxT": ((NVAR, NVAR), f32), "JGyT": ((NVAR, NVAR), f32),
    "FxT": ((8, NVAR), f32), "FyT": ((8, NVAR), f32),
    "GxT": ((8, NVAR), f32), "GyT": ((8, NVAR), f32),
    "w1n": ((2, 128, HID), bf16), "w2n": ((8, 128, HID), bf16),
    "wmu": ((8, 128, 2), bf16), "wlv": ((8, 128, 2), bf16),
    "d1n": ((1, 57, HID), bf16), "d2n": ((8, 128, HID), bf16),
    "w3c": ((8, 128, 8), bf16),
    "b1r": ((1, HID), bf16), "b2r": ((1, HID), bf16),
    "db1r": ((1, HID), bf16), "db2r": ((1, HID), bf16),
    "bmulv": ((2, 2), f32), "db3": ((8, 1), f32), "id128": ((128, 128), bf16),
    "enc_in": ((128, 2 * BL), bf16), "dec_base": ((57, BL), bf16),
    "epsT": ((2, BL), f32), "xos": ((NUM, NOBS, 2 * BL), f32),
    "ubt": ((NUM, BL), f32), "lbt": ((NUM, BL), f32),
    "bx1": ((NVAR, BL), f32), "by1": ((NVAR, BL), f32),
    "DxC": ((NVAR, BL), f32), "DyC": ((NVAR, BL), f32),
}


def _build_program():
    import concourse.bass as bass
    import concourse.mybir as mybir
    from concourse.tile import TileContext

    _install_drain_patch()
    dt = mybir.dt
    DT = {np.dtype(f32): dt.float32, np.dtype(bf16): dt.bfloat16}
    Alu = mybir.AluOpType
    Act = mybir.ActivationFunctionType

    nc = bass.Bass("TRN2", target_bir_lowering=False, debug=False, num_devices=NC)
    d = {}
    for name, (shape, dtype) in _SHAPES.items():
        d[name] = nc.dram_tensor(name, list(shape), DT[np.dtype(dtype)],
                                 kind="ExternalInput").ap()
    out_d = nc.dram_tensor("out", [2, NVAR, BL], dt.float32,
                           kind="ExternalOutput").ap()

    lnAB = float(np.log(f32(A_OBS * B_OBS)))
    lnVMAX = float(np.log(f32(V_MAX)))
    lnVMIN = float(np.log(f32(V_MIN)))
    lnAMAX = float(np.log(f32(A_MAX)))

    with TileContext(nc) as tc:
        with tc.tile_pool(name="consts", bufs=1) as cpool, \
             tc.tile_pool(name="obs", bufs=2) as opool, \
             tc.tile_pool(name="state", bufs=2) as spool:

            # ================= MLP (batch-major, N=512) =================
            with tc.tile_pool(name="wstream", bufs=5) as wpool, \
                 tc.tile_pool(name="acts", bufs=2) as apool, \
                 tc.tile_pool(name="mlpps", bufs=2, space="PSUM") as mpp, \
                 tc.tile_pool(name="trps", bufs=2, space="PSUM") as tpp:

                enc_in = cpool.tile([128, 2 * BL], dt.bfloat16, tag="c_enc_in")
                nc.sync.dma_start(out=enc_in[:], in_=d["enc_in"][:])
                id128 = cpool.tile([128, 128], dt.bfloat16, tag="c_id")
                nc.sync.dma_start(out=id128[:], in_=d["id128"][:])
                ones1 = cpool.tile([1, 128], dt.bfloat16, tag="c_ones")
                nc.vector.memset(ones1[:], 1.0)
                brow = {}
                for nm in ("b1r", "b2r", "db1r", "db2r"):
                    brow[nm] = cpool.tile([1, HID], dt.bfloat16, tag=f"c_{nm}", name=f"brow_{nm}")
                    nc.sync.dma_start(out=brow[nm][:], in_=d[nm][:])
                bmulvt = cpool.tile([2, 2], dt.float32, tag="c_bmulv")
                nc.sync.dma_start(out=bmulvt[:], in_=d["bmulv"][:])
                db3t = cpool.tile([8, 1], dt.float32, tag="c_db3")
                nc.sync.dma_start(out=db3t[:], in_=d["db3"][:])

                def dense_bm(in_fm, w_dram, n_ko, brow_t, tagp):
                    """in_fm: list of [K,128] lhsT chunk APs. Returns 8 fm
                    chunks [128,128] bf16 of relu(in @ W + b): bias via K=1
                    ones outer product, two bm psums accumulated ko-major with
                    one [K,1024] weight DMA per ko (alternating DMA queues),
                    PE transpose back to fm."""
                    ps = []
                    for no in range(2):
                        p = mpp.tile([128, 512], dt.float32, tag=f"bm_ps{no}",
                                     name=f"bmps_{tagp}{no}", bufs=1)
                        nc.tensor.matmul(p[:], ones1[:],
                                         brow_t[:, no * 512:(no + 1) * 512],
                                         start=True, stop=False)
                        ps.append(p)
                    for ko in range(n_ko):
                        wt = wpool.tile([w_dram.shape[-2], HID],
                                        dt.bfloat16, tag=f"w_{tagp}")
                        eng = nc.sync if ko % 2 == 0 else nc.gpsimd
                        eng.dma_start(out=wt[:], in_=w_dram[ko])
                        for no in range(2):
                            nc.tensor.matmul(
                                ps[no][:], in_fm[ko],
                                wt[:, no * 512:(no + 1) * 512],
                                start=False, stop=(ko == n_ko - 1))
                    fm_out = []
                    for no in range(2):
                        hbm = apool.tile([128, 512], dt.bfloat16,
                                         tag=f"hbm_{tagp}{no}")
                        nc.scalar.activation(hbm[:], ps[no][:], Act.Relu)
                        for j in range(4):
                            pst = tpp.tile([128, 128], dt.bfloat16,
                                           tag="tr_ps", bufs=3,
                                           name=f"trps_{tagp}{no}{j}")
                            nc.tensor.transpose(
                                pst[:], hbm[:, j * 128:(j + 1) * 128], id128[:])
                            fm = apool.tile([128, 128], dt.bfloat16,
                                            tag=f"fm_{tagp}{no}{j}",
                                            name=f"fm_{tagp}{no}{j}")
                            nc.vector.tensor_copy(fm[:], pst[:])
                            fm_out.append(fm)
                    return fm_out

                h1 = dense_bm([enc_in[:, 0:BL], enc_in[:, BL:2 * BL]],
                              d["w1n"], 2, brow["b1r"], "e1")
                h2 = dense_bm([h[:] for h in h1], d["w2n"], 8, brow["b2r"], "e2")

                ps_mu = tpp.tile([2, BL], dt.float32, tag="mu", bufs=1)
                ps_lv = tpp.tile([2, BL], dt.float32, tag="lv", bufs=1)
                for ko in range(8):
                    wmut = wpool.tile([128, 2], dt.bfloat16, tag="wmu")
                    (nc.sync if ko % 2 == 0 else nc.gpsimd).dma_start(
                        out=wmut[:], in_=d["wmu"][ko])
                    nc.tensor.matmul(ps_mu[:], wmut[:], h2[ko][:],
                                     start=(ko == 0), stop=(ko == 7))
                    wlvt = wpool.tile([128, 2], dt.bfloat16, tag="wlv")
                    (nc.gpsimd if ko % 2 == 0 else nc.sync).dma_start(
                        out=wlvt[:], in_=d["wlv"][ko])
                    nc.tensor.matmul(ps_lv[:], wlvt[:], h2[ko][:],
                                     start=(ko == 0), stop=(ko == 7))
                elv = apool.tile([2, BL], dt.float32, tag="elv")
                nc.scalar.activation(elv[:], ps_lv[:], Act.Exp,
                                     bias=bmulvt[:, 1:2], scale=0.5)
                mu = apool.tile([2, BL], dt.float32, tag="mu_s")
                nc.scalar.activation(mu[:], ps_mu[:], Act.Identity,
                                     bias=bmulvt[:, 0:1], scale=1.0)
                epst = cpool.tile([2, BL], dt.float32, tag="c_eps")
                nc.sync.dma_start(out=epst[:], in_=d["epsT"][:])
                dec_in = cpool.tile([57, BL], dt.bfloat16, tag="c_dec_in")
                nc.sync.dma_start(out=dec_in[:], in_=d["dec_base"][:])
                ze = apool.tile([2, BL], dt.float32, tag="ze")
                nc.vector.tensor_tensor(ze[:], elv[:], epst[:], Alu.mult)
                nc.vector.tensor_tensor(dec_in[0:2, :], ze[:], mu[:], Alu.add)

                g1 = dense_bm([dec_in[:]], d["d1n"], 1, brow["db1r"], "d1")
                g2 = dense_bm([g[:] for g in g1], d["d2n"], 8, brow["db2r"], "d2")

                ps_nn = tpp.tile([8, BL], dt.float32, tag="nn", bufs=1)
                for ko in range(8):
                    w3t = wpool.tile([128, 8], dt.bfloat16, tag="w3")
                    (nc.sync if ko % 2 == 0 else nc.gpsimd).dma_start(
                        out=w3t[:], in_=d["w3c"][ko])
                    nc.tensor.matmul(ps_nn[:], w3t[:], g2[ko][:],
                                     start=(ko == 0), stop=(ko == 7))
                nn = cpool.tile([8, BL], dt.float32, tag="c_nn")
                nc.scalar.activation(nn[:], ps_nn[:], Act.Identity,
                                     bias=db3t[:], scale=1.0)

            # ================= setup + ADMM loop =================
            with tc.tile_pool(name="fwdps", bufs=4, space="PSUM") as fps, \
                 tc.tile_pool(name="chainps", bufs=4, space="PSUM") as cps:

                def ctile(name, eng=None):
                    shape = list(_SHAPES[name][0])
                    dtype = DT[np.dtype(_SHAPES[name][1])]
                    t = cpool.tile(shape, dtype, tag=f"c_{name}", name=f"ct_{name}")
                    (eng or nc.sync).dma_start(out=t[:], in_=d[name][:])
                    return t

                PTt = ctile("PT"); PdTt = ctile("PdT"); PddTt = ctile("PddT")
                Loxt = ctile("Lox"); Loyt = ctile("Loy")
                Lat = ctile("La"); Lvt = ctile("Lv"); Llt = ctile("Ll")
                Jxt = ctile("Jx"); Jyt = ctile("Jy")
                JGxTt = ctile("JGxT"); JGyTt = ctile("JGyT")
                FxTt = ctile("FxT"); FyTt = ctile("FyT")
                GxTt = ctile("GxT"); GyTt = ctile("GyT")
                xost = ctile("xos", eng=nc.gpsimd)
                ubtt = ctile("ubt"); lbtt = ctile("lbt")
                bx1t = ctile("bx1"); by1t = ctile("by1")
                DxCt = ctile("DxC"); DyCt = ctile("DyC")

                cbias = cpool.tile([128, 6], dt.float32, tag="c_cbias")
                nc.vector.memset(cbias[:, 0:1], EPS_LN)
                nc.vector.memset(cbias[:, 1:2], lnAB)
                nc.vector.memset(cbias[:, 2:3], -lnVMAX)
                nc.vector.memset(cbias[:, 3:4], lnVMIN)
                nc.vector.memset(cbias[:, 4:5], -lnAMAX)

                # prim0 / Dx2 / Dy2
                ps0 = cps.tile([NVAR, BL], dt.float32, tag="chain")
                prim0 = cpool.tile([NVAR, 2 * BL], dt.float32, tag="c_prim0")
                nc.tensor.matmul(ps0[:], FxTt[:], nn[:], start=True, stop=True)
                nc.vector.tensor_tensor(prim0[:, 0:BL], ps0[:], bx1t[:], Alu.add)
                ps1 = cps.tile([NVAR, BL], dt.float32, tag="chain")
                nc.tensor.matmul(ps1[:], FyTt[:], nn[:], start=True, stop=True)
                nc.vector.tensor_tensor(prim0[:, BL:2 * BL], ps1[:], by1t[:],
                                        Alu.add)
                Dx2 = cpool.tile([NVAR, BL], dt.float32, tag="c_Dx2")
                ps2 = cps.tile([NVAR, BL], dt.float32, tag="chain")
                nc.tensor.matmul(ps2[:], GxTt[:], nn[:], start=True, stop=True)
                nc.vector.tensor_tensor(Dx2[:], ps2[:], DxCt[:], Alu.add)
                Dy2 = cpool.tile([NVAR, BL], dt.float32, tag="c_Dy2")
                ps3 = cps.tile([NVAR, BL], dt.float32, tag="chain")
                nc.tensor.matmul(ps3[:], GyTt[:], nn[:], start=True, stop=True)
                nc.vector.tensor_tensor(Dy2[:], ps3[:], DyCt[:], Alu.add)

                # recentering: xy0 (scaled), shifted obstacle / lane tiles
                ps_xy0 = fps.tile([NUM, 2 * BL], dt.float32, tag="fwd")
                nc.tensor.matmul(ps_xy0[:], PTt[:], prim0[:],
                                 start=True, stop=True)
                xy0s = cpool.tile([NUM, 2 * BL], dt.float32, tag="c_xy0s")
                nc.scalar.activation(xy0s[:, 0:BL], ps_xy0[:, 0:BL], Act.Copy,
                                     scale=float(B_OBS))
                nc.scalar.activation(xy0s[:, BL:2 * BL], ps_xy0[:, BL:2 * BL],
                                     Act.Copy, scale=float(A_OBS))
                xo_r = cpool.tile([NUM, NOBS, 2 * BL], dt.bfloat16, tag="c_xo_r")
                nc.vector.tensor_tensor(
                    xo_r[:], xost[:],
                    xy0s[:, None, :].to_broadcast((NUM, NOBS, 2 * BL)),
                    Alu.subtract)
                ub_r = cpool.tile([NUM, BL], dt.float32, tag="c_ub_r")
                nc.vector.tensor_tensor(ub_r[:], ubtt[:], xy0s[:, BL:2 * BL],
                                        Alu.subtract)
                lb_r = cpool.tile([NUM, BL], dt.float32, tag="c_lb_r")
                nc.vector.tensor_tensor(lb_r[:], lbtt[:], xy0s[:, BL:2 * BL],
                                        Alu.subtract)

                DDx = cpool.tile([NVAR, BL], dt.float32, tag="c_DDx")
                nc.vector.tensor_tensor(DDx[:], Dx2[:], prim0[:, 0:BL],
                                        Alu.subtract)
                DDy = cpool.tile([NVAR, BL], dt.float32, tag="c_DDy")
                nc.vector.tensor_tensor(DDy[:], Dy2[:], prim0[:, BL:2 * BL],
                                        Alu.subtract)

                lx = spool.tile([NVAR, BL], dt.float32, tag="lx")
                ly = spool.tile([NVAR, BL], dt.float32, tag="ly")
                nc.vector.memset(lx[:], 0.0)
                nc.vector.memset(ly[:], 0.0)
                prim = prim0
                dprim = None

                # ---------- ADMM loop ----------
                for it in range(MAXITER):
                    ps_xyd = fps.tile([NUM, 2 * BL], dt.float32, tag="fwd")
                    nc.tensor.matmul(ps_xyd[:], PdTt[:], prim[:],
                                     start=True, stop=True)
                    ps_xydd = fps.tile([NUM, 2 * BL], dt.float32, tag="fwd")
                    nc.tensor.matmul(ps_xydd[:], PddTt[:], prim[:],
                                     start=True, stop=True)

                    # scaled recentered positions, bf16 [100, 256]
                    xys = opool.tile([NUM, 2 * BL], dt.bfloat16, tag="xys")
                    if it == 0:
                        nc.vector.memset(xys[:], 0.0)
                    else:
                        ps_xy = fps.tile([NUM, 2 * BL], dt.float32, tag="fwd")
                        nc.tensor.matmul(ps_xy[:], PTt[:], dprim[:],
                                         start=True, stop=True)
                        nc.scalar.activation(xys[:, 0:BL], ps_xy[:, 0:BL],
                                             Act.Copy, scale=float(B_OBS))
                        nc.scalar.activation(xys[:, BL:2 * BL],
                                             ps_xy[:, BL:2 * BL],
                                             Act.Copy, scale=float(A_OBS))

                    # obstacles: wc' = xys - xo_r (bf16, recentered),
                    # processed in two halves so stages pipeline
                    HOB = NOBS // 2
                    wcws = opool.tile([NUM, NOBS, 2 * BL], dt.bfloat16,
                                      tag="wcws")
                    sq = opool.tile([NUM, NOBS, 2 * BL], dt.bfloat16, tag="sq")
                    r2 = opool.tile([NUM, NOBS + 2, BL], dt.bfloat16, tag="r2")
                    for hh in range(2):
                        oh = slice(hh * HOB, (hh + 1) * HOB)
                        nc.vector.tensor_tensor(
                            wcws[:, oh, :],
                            xys[:, None, :].to_broadcast((NUM, HOB, 2 * BL)),
                            xo_r[:, oh, :], Alu.subtract)
                        nc.vector.tensor_tensor(sq[:, oh, 0:BL],
                                                wcws[:, oh, 0:BL],
                                                wcws[:, oh, 0:BL], Alu.mult)
                        nc.scalar.activation(sq[:, oh, BL:2 * BL],
                                             wcws[:, oh, BL:2 * BL], Act.Square)
                        nc.vector.tensor_tensor(r2[:, oh, :], sq[:, oh, 0:BL],
                                                sq[:, oh, BL:2 * BL], Alu.add)
                    # velocity / accel squares straight from PSUM
                    qd = opool.tile([NUM, 4 * BL], dt.bfloat16, tag="qd")
                    nc.scalar.activation(qd[:, 0:2 * BL], ps_xyd[:], Act.Square)
                    nc.scalar.activation(qd[:, 2 * BL:4 * BL], ps_xydd[:],
                                         Act.Square)
                    nc.vector.tensor_tensor(r2[:, NOBS, :], qd[:, 0:BL],
                                            qd[:, BL:2 * BL], Alu.add)
                    nc.vector.tensor_tensor(r2[:, NOBS + 1, :],
                                            qd[:, 2 * BL:3 * BL],
                                            qd[:, 3 * BL:4 * BL], Alu.add)

                    lnt = opool.tile([NUM, NOBS + 2, BL], dt.bfloat16, tag="lnt")
                    qob = opool.tile([NUM, NOBS, BL], dt.bfloat16, tag="qob")
                    em = opool.tile([NUM, NOBS, BL], dt.bfloat16, tag="em")
                    m1 = opool.tile([NUM, NOBS, BL], dt.bfloat16, tag="m1")
                    uv = opool.tile([NUM, NOBS, 2 * BL], dt.bfloat16, tag="uv")
                    for hh in range(2):
                        oh = slice(hh * HOB, (hh + 1) * HOB)
                        nc.scalar.activation(lnt[:, oh, :], r2[:, oh, :],
                                             Act.Ln, bias=cbias[0:NUM, 0:1])
                        # m1-1 = max(exp(-ln/2 + lnAB), 1) - 1 (clamps on DVE)
                        nc.vector.tensor_scalar(qob[:, oh, :], lnt[:, oh, :],
                                                -0.5, lnAB, Alu.mult, Alu.add)
                        nc.scalar.activation(em[:, oh, :], qob[:, oh, :],
                                             Act.Exp)
                        nc.vector.tensor_scalar(m1[:, oh, :], em[:, oh, :],
                                                1.0, 1.0, Alu.max, Alu.subtract)
                        nc.vector.tensor_tensor(uv[:, oh, 0:BL], m1[:, oh, :],
                                                wcws[:, oh, 0:BL], Alu.mult)
                        nc.vector.tensor_tensor(uv[:, oh, BL:2 * BL],
                                                m1[:, oh, :],
                                                wcws[:, oh, BL:2 * BL], Alu.mult)
                    nc.scalar.activation(lnt[:, NOBS:NOBS + 2, :],
                                         r2[:, NOBS:NOBS + 2, :],
                                         Act.Ln, bias=cbias[0:NUM, 0:1])

                    # velocity: gv = max(min(Vmax/rv,1), Vmin/rv)
                    qv = opool.tile([NUM, BL], dt.bfloat16, tag="qv")
                    nc.scalar.activation(qv[:], lnt[:, NOBS, :], Act.Relu,
                                         bias=cbias[0:NUM, 2:3], scale=0.5)
                    gva = opool.tile([NUM, BL], dt.bfloat16, tag="gva")
                    nc.scalar.activation(gva[:], qv[:], Act.Exp, scale=-1.0)
                    gvb = opool.tile([NUM, BL], dt.bfloat16, tag="gvb")
                    nc.scalar.activation(gvb[:], lnt[:, NOBS, :], Act.Exp,
                                         bias=cbias[0:NUM, 3:4], scale=-0.5)
                    gv = opool.tile([NUM, BL], dt.bfloat16, tag="gv")
                    nc.vector.tensor_tensor(gv[:], gva[:], gvb[:], Alu.max)
                    uvd = opool.tile([NUM, 2, BL], dt.bfloat16, tag="uvd")
                    nc.vector.scalar_tensor_tensor(
                        uvd[:], gv[:, None, :].to_broadcast((NUM, 2, BL)),
                        1.0, ps_xyd[:].rearrange("p (c b) -> p c b", c=2),
                        Alu.subtract, Alu.mult)

                    # accel: ga = min(Amax/ra, 1)
                    qa = opool.tile([NUM, BL], dt.bfloat16, tag="qa")
                    nc.scalar.activation(qa[:], lnt[:, NOBS + 1, :], Act.Relu,
                                         bias=cbias[0:NUM, 4:5], scale=0.5)
                    ga = opool.tile([NUM, BL], dt.bfloat16, tag="ga")
                    nc.scalar.activation(ga[:], qa[:], Act.Exp, scale=-1.0)
                    uad = opool.tile([NUM, 2, BL], dt.bfloat16, tag="uad")
                    nc.vector.scalar_tensor_tensor(
                        uad[:], ga[:, None, :].to_broadcast((NUM, 2, BL)),
                        1.0, ps_xydd[:].rearrange("p (c b) -> p c b", c=2),
                        Alu.subtract, Alu.mult)

                    # lane (recentered, A-scaled)
                    c1 = opool.tile([NUM, BL], dt.float32, tag="c1")
                    nc.vector.tensor_tensor(c1[:], xys[:, BL:2 * BL], lb_r[:],
                                            Alu.max)
                    c2 = opool.tile([NUM, BL], dt.float32, tag="c2")
                    nc.vector.tensor_tensor(c2[:], c1[:], ub_r[:], Alu.min)
                    elane = opool.tile([NUM, BL], dt.bfloat16, tag="elane")
                    nc.vector.tensor_tensor(elane[:], xys[:, BL:2 * BL], c2[:],
                                            Alu.subtract)

                    # T chains
                    ps_tx = cps.tile([NVAR, BL], dt.float32, tag="chain")
                    for o in range(NOBS):
                        nc.tensor.matmul(ps_tx[:], Loxt[:], uv[:, o, 0:BL],
                                         start=(o == 0), stop=False)
                    nc.tensor.matmul(ps_tx[:], Lvt[:], uvd[:, 0, :],
                                     start=False, stop=False)
                    nc.tensor.matmul(ps_tx[:], Lat[:], uad[:, 0, :],
                                     start=False, stop=True)
                    ps_ty = cps.tile([NVAR, BL], dt.float32, tag="chain")
                    for o in range(NOBS):
                        nc.tensor.matmul(ps_ty[:], Loyt[:], uv[:, o, BL:2 * BL],
                                         start=(o == 0), stop=False)
                    nc.tensor.matmul(ps_ty[:], Lvt[:], uvd[:, 1, :],
                                     start=False, stop=False)
                    nc.tensor.matmul(ps_ty[:], Lat[:], uad[:, 1, :],
                                     start=False, stop=False)
                    nc.tensor.matmul(ps_ty[:], Llt[:], elane[:],
                                     start=False, stop=True)

                    # z = lx + 2*T  (J z = J lx_new + J T), straight from PSUM
                    z_x = opool.tile([NVAR, BL], dt.float32, tag="z_x")
                    nc.vector.scalar_tensor_tensor(
                        z_x[:], ps_tx[:], 2.0, lx[:], Alu.mult, Alu.add)
                    z_y = opool.tile([NVAR, BL], dt.float32, tag="z_y")
                    nc.vector.scalar_tensor_tensor(
                        z_y[:], ps_ty[:], 2.0, ly[:], Alu.mult, Alu.add)
                    lx_n = spool.tile([NVAR, BL], dt.float32, tag="lx")
                    nc.vector.tensor_tensor(lx_n[:], lx[:], ps_tx[:], Alu.add)
                    ly_n = spool.tile([NVAR, BL], dt.float32, tag="ly")
                    nc.vector.tensor_tensor(ly_n[:], ly[:], ps_ty[:], Alu.add)

                    ps_sx = cps.tile([NVAR, BL], dt.float32, tag="chain")
                    nc.tensor.matmul(ps_sx[:], Jxt[:], z_x[:],
                                     start=True, stop=False)
                    nc.tensor.matmul(ps_sx[:], JGxTt[:], prim[:, 0:BL],
                                     start=False, stop=True)
                    ps_sy = cps.tile([NVAR, BL], dt.float32, tag="chain")
                    nc.tensor.matmul(ps_sy[:], Jyt[:], z_y[:],
                                     start=True, stop=False)
                    nc.tensor.matmul(ps_sy[:], JGyTt[:], prim[:, BL:2 * BL],
                                     start=False, stop=True)

                    prim_n = spool.tile([NVAR, 2 * BL], dt.float32, tag="prim")
                    nc.vector.tensor_tensor(prim_n[:, 0:BL], ps_sx[:], Dx2[:],
                                            Alu.add)
                    nc.vector.tensor_tensor(prim_n[:, BL:2 * BL], ps_sy[:],
                                            Dy2[:], Alu.add)

                    if it + 1 < MAXITER:
                        dprim_n = spool.tile([NVAR, 2 * BL], dt.float32,
                                             tag="dprim")
                        nc.vector.tensor_tensor(dprim_n[:, 0:BL], ps_sx[:],
                                                DDx[:], Alu.add)
                        nc.vector.tensor_tensor(dprim_n[:, BL:2 * BL], ps_sy[:],
                                                DDy[:], Alu.add)
                        dprim = dprim_n
                    prim, lx, ly = prim_n, lx_n, ly_n

                # ---------- output ----------
                nc.sync.dma_start(out=out_d[0], in_=prim[:, 0:BL])
                nc.sync.dma_start(out=out_d[1], in_=prim[:, BL:2 * BL])

    _split_multi_waits(nc)
    return nc


# ---------------------------------------------------------------------------
# Entry point
# ---------------------------------------------------------------------------

def kernel(**inputs):
    from concourse.bass_utils import run_bass_kernel_spmd

    shared, per_core = _host_prep(inputs)
    nc = _build_program()
    in_maps = [{**shared, **pc} for pc in per_core]
    res = run_bass_kernel_spmd(nc, in_maps, list(range(NC)))
    out = np.empty((B, 2 * NVAR), f32)
    for c in range(NC):
        r = res.results[c]["out"]
        out[c * BL:(c + 1) * BL, 0:NVAR] = r[0].T
        out[c * BL:(c + 1) * BL, NVAR:2 * NVAR] = r[1].T
    return out
